# revision 27
# baseline (speedup 1.0000x reference)
"""Distributed Trainium2 kernel for nn_ApaBlock (8 NeuronCores, data-parallel).

Architecture (per core, batch shard of 256 rows):
  Z = relu(X @ W1 + b1)                               (TensorE + DVE/ACT)
  ZT_bcast[p, t, b] = Z^T[t, b]  (replicated over p)  (broadcast DMA, once)
  scan over 8 ranks:
    U^T_t = ZiT * ZT_bcast[t]      (DVE bf16 2x mode, chunked)
    G^T  += P[:,t,:]^T @ U^T_t     (128 accumulating 256-wide matmuls)
    batch stats via ACT accum_out -> PE-transpose to row layout
    cross-core AllGather (sync-BN) in [rows, 128] layout; reduce via
    1/B-scaled ones matmuls (f32, no cast); coeffs with fused Rsqrt
    Zi+1^T = a*G^T + c  fused into the PSUM evacuation
  Y = BN(sum Zi/8) via closed-form global sums (no extra sync);
  out = relu(relu(Y@W3+b3) + relu(X@W2+b2))

v2 perf notes (vs 417us baseline):
  - ztb broadcast is issued as early as possible and with queue priority
    over the P stream; rank-0 U chunks are 16-aligned so they chase the
    broadcast DMA chunk by chunk.
  - The exposed BN-sync window (~15us) gets "warm-keeper" dummy matmuls
    so the PE HAM clock gate never sees a >3.4us idle window; the rank's
    matmul stream then restarts at full clock instead of 1.2GHz.
  - Reduce matmuls run on f32 gather data directly with 1/B pre-scaled
    ones (drops a cast + a scale op); coeff chain uses ACT Rsqrt and
    fused tensor_scalar ops.
  - yt accumulation runs on GPSIMD so DVE is left for the U build
    (DVE is the in-rank pacer at ~17.1us vs PE 16.8us).
  - relu(X@W2+b2) is computed inside rank-0's sync window (free PE time).
"""

import os
import sys
import types

if "/opt/trn_rl_repo" not in sys.path:
    sys.path.insert(0, "/opt/trn_rl_repo")

import numpy as np
import ml_dtypes

N_CORES = 8
B, IN, H, OUT, RANK = 2048, 256, 128, 128, 8
BS = B // N_CORES  # 256 rows per core
NBT = BS // 128  # 2 b-tiles per core
EPS = 1e-5
QK = H * H  # 16384
NCH = 8  # P DMA chunks per rank
TCH = H // NCH  # t's (q-planes) per P chunk (16)
# U-build chunk sizes: 16-aligned so every chunk reads exactly one ztb
# broadcast tile (fine-grained DMA deps let rank 0 chase the broadcast).
# Small chunks LAST: G completes right behind the final DVE build (the
# trailing matmul tail is 8 instead of 16 matmuls).
UCH = [16, 16, 16, 16, 16, 16, 16, 8, 8]  # sum 128
UMAX = max(UCH)

_cache = {}


def _ensure_axon_hooks_shim():
    """bass_utils imports antenv.axon_hooks when BASS_TRACE is set; the agent
    image lacks it. Provide a null shim so tracing degrades gracefully."""
    try:
        import antenv.axon_hooks  # noqa: F401
        return
    except ImportError:
        pass
    try:
        import antenv  # noqa: F401
    except ImportError:
        return
    mod = types.ModuleType("antenv.axon_hooks")
    _state = {"hook": None}
    mod.set_axon_ntff_profile_hook = lambda h: _state.__setitem__("hook", h)
    mod.get_axon_ntff_profile_hook = lambda: _state["hook"]
    sys.modules["antenv.axon_hooks"] = mod


def _build():
    from concourse import bacc, mybir, tile

    f32 = mybir.dt.float32
    bf16 = mybir.dt.bfloat16
    FT = mybir.ActivationFunctionType
    AL = mybir.AluOpType

    nc = bacc.Bacc("TRN2", target_bir_lowering=False, debug=False,
                   num_devices=N_CORES)

    XTd = nc.declare_dram_parameter("XT", [2, 128, BS], bf16, isOutput=False)
    Pd = nc.declare_dram_parameter("P", [RANK, H, QK], bf16, isOutput=False)
    W1d = nc.declare_dram_parameter("W1", [2, 128, H], bf16, isOutput=False)
    W2d = nc.declare_dram_parameter("W2", [2, 128, OUT], bf16, isOutput=False)
    W3d = nc.declare_dram_parameter("W3", [H, OUT], bf16, isOutput=False)
    B1d = nc.declare_dram_parameter("b1b", [128, H], f32, isOutput=False)
    B2d = nc.declare_dram_parameter("b2b", [128, OUT], f32, isOutput=False)
    B3d = nc.declare_dram_parameter("b3b", [128, OUT], f32, isOutput=False)
    BNCd = nc.declare_dram_parameter("bnc", [H, 6], f32, isOutput=False)
    IDd = nc.declare_dram_parameter("ident", [128, 128], bf16, isOutput=False)
    IDFd = nc.declare_dram_parameter("identf", [128, 128], f32,
                                     isOutput=False)
    OUTd = nc.declare_dram_parameter("out", [BS, OUT], f32, isOutput=True)

    rg = [list(range(N_CORES))]
    CW = TCH * 128  # P columns per chunk (2048)

    with tile.TileContext(nc) as tc:
        with (
            tc.tile_pool(name="const", bufs=1) as cpool,
            tc.tile_pool(name="ppool", bufs=1) as ppool,
            tc.tile_pool(name="upool", bufs=1) as upool,
            tc.tile_pool(name="zit", bufs=2) as zitpool,
            tc.tile_pool(name="small", bufs=2) as spool,
            tc.tile_pool(name="psg", bufs=2, space="PSUM") as psg,
            tc.tile_pool(name="psmm", bufs=2, space="PSUM") as psmm,
            tc.tile_pool(name="pstr", bufs=1, space="PSUM") as pstr,
            tc.tile_pool(name="psd", bufs=1, space="PSUM") as psd,
            tc.tile_pool(name="dram", bufs=4, space="DRAM") as dpool,
        ):
            # Early dummy collectives: the first absorbs cross-core launch
            # skew + ncfw first-call overhead; the second absorbs the slow
            # second-call path so rank-0's real sync runs at steady cost.
            dz = cpool.tile([1, 2 * H], f32, tag="dz")
            nc.vector.memset(dz[:], 0.0)
            dsrc = dpool.tile([1, 2 * H], f32, tag="ccsrcd")
            ddst = dpool.tile([N_CORES, 2 * H], f32, tag="ccdstd")
            nc.scalar.dma_start(dsrc[:], dz[:])
            nc.gpsimd.collective_compute(
                "AllGather", AL.bypass, replica_groups=rg,
                ins=[dsrc.opt()], outs=[ddst.opt()],
            )

            # ---- sync queue: xt (Z dep) -> p0,p1 -> (after dzt) ztb 0..3
            #      -> p2..p7.  scalar queue: w1/b1b/ident (Z deps) ->
            #      dzt-out -> ztb 4..7 -> remaining constants.
            xt = cpool.tile([128, 2 * BS], bf16, tag="xt")
            for c in range(2):
                nc.sync.dma_start(xt[:, c * BS:(c + 1) * BS], XTd[c])

            w1 = cpool.tile([128, 2 * H], bf16, tag="w1")
            for c in range(2):
                nc.scalar.dma_start(w1[:, c * H:(c + 1) * H], W1d[c])
            b1b = cpool.tile([128, H], f32, tag="b1b")
            nc.scalar.dma_start(b1b[:], B1d[:])
            ident = cpool.tile([128, 128], bf16, tag="ident")
            nc.scalar.dma_start(ident[:], IDd[:])
            identf = cpool.tile([128, 128], f32, tag="identf")
            nc.scalar.dma_start(identf[:], IDFd[:])

            p_ch = [ppool.tile([128, CW], bf16, tag=f"p{c}", name=f"p{c}")
                    for c in range(NCH)]
            for c in range(2):
                nc.sync.dma_start(p_ch[c][:], Pd[0][:, c * CW:(c + 1) * CW])

            ones11 = cpool.tile([1, 1], f32, tag="ones11")
            nc.vector.memset(ones11[:], 1.0)
            # cross-core reduce weights with 1/B folded in (st cols become
            # E[x], E[x^2] directly; rank-7 sums become raw/B "primes")
            onesB = cpool.tile([N_CORES, 1], f32, tag="onesB")
            nc.vector.memset(onesB[:], 1.0 / B)
            epsc = cpool.tile([H, 1], f32, tag="epsc")
            nc.vector.memset(epsc[:], EPS)
            epsr = cpool.tile([1, 1], f32, tag="epsr")
            nc.vector.memset(epsr[:], EPS)

            yt = cpool.tile([H, BS], f32, tag="yt")  # Y^T accumulator
            nc.vector.memset(yt[:], 0.0)

            # ---------------- Z = relu(X@W1 + b1) ----------------
            zb = cpool.tile([128, 2 * H], bf16, tag="zb")  # Z, b-partition
            for bt in range(NBT):
                ps = psmm.tile([128, H], f32, tag="mm")
                for c in range(2):
                    nc.tensor.matmul(
                        ps[:],
                        lhsT=xt[:, c * BS + bt * 128: c * BS + (bt + 1) * 128],
                        rhs=w1[:, c * H:(c + 1) * H],
                        start=(c == 0), stop=(c == 1),
                    )
                t0 = spool.tile([128, H], f32, tag="ztmp")
                nc.vector.tensor_tensor(t0[:], ps[:], b1b[:], AL.add)
                nc.scalar.activation(zb[:, bt * H:(bt + 1) * H], t0[:],
                                     FT.Relu)

            # Z^T (q-part, b): initial Zi^T, and the source for ZT_bcast
            zt = cpool.tile([H, BS], bf16, tag="zt")
            for bt in range(NBT):
                pst = pstr.tile([128, 128], bf16, tag="tr")
                nc.tensor.transpose(pst[:],
                                    zb[:, bt * H:(bt + 1) * H], ident[:])
                nc.scalar.activation(zt[:, bt * 128:(bt + 1) * 128],
                                     pst[:], FT.Copy)

            # ZT_bcast[p, (t, b)] = ZT[t, b]: bounce ZT to DRAM, then
            # broadcast-read it back into all 128 partitions, chunked on
            # both queues so rank-0 U-builds chase the chunks.
            dzt = dpool.tile([H, BS], bf16, tag="dzt")
            nc.scalar.dma_start(dzt[:], zt[:])
            ztbs = [cpool.tile([128, TCH * BS], bf16, tag=f"ztb{c}",
                               name=f"ztb{c}")
                    for c in range(NCH)]
            ztb3s = [zc[:].rearrange("p (t b) -> p t b", b=BS)
                     for zc in ztbs]
            # flat contiguous source view: one 8KB descriptor per dest
            # partition instead of 16 strided 512B ones (the [t, b] rows of
            # dzt are contiguous in DRAM) -> ~2x faster broadcast.
            # Interleave the remaining rank-0 P chunks between the even ztb
            # chunks so rank-0's matmuls get P roughly when the matching
            # ztb tile lands (both gate chunk-k progress).
            dztf = dzt[:].rearrange("(o t) b -> o (t b)", o=1)

            def ztb_load(c):
                src = dztf[0:1, c * TCH * BS:(c + 1) * TCH * BS].rearrange(
                    "o n -> o n").broadcast_to((128, TCH * BS))
                eng = nc.sync if c % 2 == 0 else nc.scalar
                eng.dma_start(ztbs[c][:, :], src)

            for c in range(1, NCH, 2):
                ztb_load(c)
            ztb_load(0)
            nc.sync.dma_start(p_ch[2][:], Pd[0][:, 2 * CW:3 * CW])
            ztb_load(2)
            nc.sync.dma_start(p_ch[3][:], Pd[0][:, 3 * CW:4 * CW])
            ztb_load(4)
            nc.sync.dma_start(p_ch[4][:], Pd[0][:, 4 * CW:5 * CW])
            ztb_load(6)
            for c in range(5, NCH):
                nc.sync.dma_start(p_ch[c][:], Pd[0][:, c * CW:(c + 1) * CW])

            # non-critical constants (needed from rank-0 sync onwards)
            w2 = cpool.tile([128, 2 * OUT], bf16, tag="w2")
            for c in range(2):
                nc.scalar.dma_start(w2[:, c * OUT:(c + 1) * OUT], W2d[c])
            w3 = cpool.tile([H, OUT], bf16, tag="w3")
            nc.scalar.dma_start(w3[:], W3d[:])
            b2b = cpool.tile([128, OUT], f32, tag="b2b")
            b3b = cpool.tile([128, OUT], f32, tag="b3b")
            nc.scalar.dma_start(b2b[:], B2d[:])
            nc.scalar.dma_start(b3b[:], B3d[:])
            # bnc cols: [gz, bz, gy, by, -gz, -gy]
            bnc = cpool.tile([H, 6], f32, tag="bnc")
            nc.scalar.dma_start(bnc[:], BNCd[:])

            # ---------------- scan over ranks ----------------
            zit = zt
            gpsum = None
            pend_yt = []
            stg = a_ap = c_ap = None
            for r in range(RANK):
                if r > 0:
                    p_ch = [ppool.tile([128, CW], bf16, tag=f"p{c}",
                                       name=f"p{c}")
                            for c in range(NCH)]
                    for c in range(NCH):
                        nc.sync.dma_start(p_ch[c][:],
                                          Pd[r][:, c * CW:(c + 1) * CW])

                gpsum = psg.tile([128, BS], f32, tag="g")
                t0c = 0
                for j, csz in enumerate(UCH):
                    # U^T chunk: ut[p, t, b] = ZiT[p, b] * ZT[t, b]
                    zin = zit[:].rearrange("p (o b) -> p o b", o=1
                                           ).broadcast_to((128, csz, BS))
                    ut = upool.tile([128, UMAX * BS], bf16, tag=f"u{j % 4}",
                                    name=f"u{j % 4}")
                    ut3 = ut[:, 0:csz * BS].rearrange("p (t b) -> p t b",
                                                      b=BS)
                    zc, zo = t0c // TCH, t0c % TCH
                    # last chunk on GPSIMD: it is ready early (built in
                    # parallel from zit) and shortens the DVE critical
                    # path, which paces the rank stream
                    eng_u = (nc.gpsimd if j == len(UCH) - 1 else nc.vector)
                    eng_u.tensor_tensor(
                        ut3, zin, ztb3s[zc][:, zo:zo + csz, :], AL.mult)
                    for i in range(csz):
                        t = t0c + i
                        pc, pi = t // TCH, t % TCH
                        nc.tensor.matmul(
                            gpsum[:],
                            lhsT=p_ch[pc][:, pi * 128:(pi + 1) * 128],
                            rhs=ut3[:, i, :],
                            start=(t == 0), stop=(t == H - 1),
                        )
                    t0c += csz

                # batch stats straight from PSUM via ACT accum_out
                last = (r == RANK - 1)
                stw = 8 if last else 2
                stl = spool.tile([H, stw], f32, tag=f"stl{stw}")
                if last:
                    nc.vector.memset(stl[:], 0.0)
                scr = spool.tile([H, BS], bf16, tag="scr")
                if last:
                    gt = spool.tile([H, BS], bf16, tag="gt")
                    nc.scalar.activation(gt[:], gpsum[:], FT.Copy,
                                         accum_out=stl[:, 0:1])
                else:
                    # S1 on DVE (parallel with ACT's Square pass)
                    nc.vector.tensor_reduce(stl[:, 0:1], gpsum[:],
                                            mybir.AxisListType.X, AL.add)
                nc.scalar.activation(scr[:], gpsum[:], FT.Square,
                                     accum_out=stl[:, 1:2])
                if last:
                    # piggyback Y-BN inputs on the final sync: with
                    # R = sum_{i<8} Zi (= yt now) and Zi8 = a*G + c,
                    # SumY and SumY^2 expand in closed form from
                    # [S1G, S2G, S1R, S2R, Sum(R*G)] -- no 9th sync.
                    nc.scalar.activation(scr[:], yt[:], FT.Copy,
                                         accum_out=stl[:, 2:3])
                    nc.scalar.activation(scr[:], yt[:], FT.Square,
                                         accum_out=stl[:, 3:4])
                    scry2 = spool.tile([H, BS], bf16, tag="scry2")
                    nc.vector.tensor_tensor(scry2[:], yt[:], gt[:], AL.mult)
                    nc.scalar.activation(scr[:], scry2[:], FT.Copy,
                                         accum_out=stl[:, 4:5])

                # flatten stats into a single partition-0 row [1, stw*128]
                # (the cross-core bounce then needs only 1 DMA descriptor
                # instead of 16 partition-group descriptors = ~6us saved).
                # f32 lhsT: the fp32 double-pass on these tiny N=128
                # matmuls is cheaper than a serial cast hop.
                strow = spool.tile([1, 8 * 128], f32, tag="strow",
                                   bufs=1)
                for g in range((stw + 3) // 4):
                    ncol = min(4, stw - 4 * g)
                    pstt = pstr.tile([1, 512], f32, tag="trs", name="pstt")
                    for s4 in range(ncol):
                        s = 4 * g + s4
                        nc.tensor.matmul(pstt[0:1, s4 * 128:(s4 + 1) * 128],
                                         lhsT=stl[:, s:s + 1], rhs=identf[:],
                                         start=True, stop=True)
                    nc.scalar.activation(
                        strow[0:1, g * 512:g * 512 + ncol * 128],
                        pstt[0:1, 0:ncol * 128], FT.Copy)

                if r == 0:
                    # relu(X@W2+b2): real work to fill rank-0's sync window
                    r2rs = []
                    for bt in range(NBT):
                        psB = psmm.tile([128, OUT], f32, tag="mm")
                        for c in range(2):
                            nc.tensor.matmul(
                                psB[:],
                                lhsT=xt[:, c * BS + bt * 128:
                                        c * BS + (bt + 1) * 128],
                                rhs=w2[:, c * OUT:(c + 1) * OUT],
                                start=(c == 0), stop=(c == 1),
                            )
                        r2 = spool.tile([128, OUT], f32, tag="r2")
                        nc.vector.tensor_tensor(r2[:], psB[:], b2b[:],
                                                AL.add)
                        r2r = spool.tile([128, OUT], f32, tag=f"r2r{bt}")
                        nc.scalar.activation(r2r[:], r2[:], FT.Relu)
                        r2rs.append(r2r)

                # warm-keeper chain: keep the PE HAM activity window fed
                # through the collective flight so the next rank's matmuls
                # restart at full clock. Gated on scr (right after the
                # rank's real matmuls); ~36x N=512 covers the window.
                if r < RANK - 1:
                    dps = psd.tile([128, 512], f32, tag="warm")
                    nc.tensor.matmul(dps[0:1, 0:256], lhsT=ident[:, 0:1],
                                     rhs=scr[:, 0:BS], start=True,
                                     stop=True)
                    for _ in range(35):
                        nc.tensor.matmul(dps[0:1, :], lhsT=ident[:, 0:1],
                                         rhs=ztbs[0][:, 0:512], start=True,
                                         stop=True)

                # ---- cross-core AllGather of row stats + coeffs ----
                a_ap, c_ap, stg = _bn_sync(nc, tc, dpool, spool, pstr, psd,
                                           strow, stw, bnc, onesB, ident,
                                           ztbs[0], epsc)

                # BN apply fused into the PSUM evacuation:
                # Zi+1^T = a*G^T + c  (then first U chunks; yt flush later)
                zit_next = zitpool.tile([H, BS], bf16, tag="zit")
                nc.vector.tensor_scalar(zit_next[:], gpsum[:], a_ap, c_ap,
                                        AL.mult, AL.add)
                zit = zit_next
                if r < RANK - 1:
                    pend_yt.append(zit_next)
                else:
                    nc.vector.tensor_tensor(yt[:], yt[:], zit_next[:],
                                            AL.add)

                # deferred yt += Zi_r on GPSIMD (keeps DVE free for U build)
                while pend_yt and r < RANK - 1:
                    nc.gpsimd.tensor_tensor(yt[:], yt[:], pend_yt.pop()[:],
                                            AL.add)

            # ------- Y BN from closed-form global sums (no extra sync) ----
            # stg cols (all already scaled by 1/B): [S1G', S2G', S1R',
            # S2R', SX'].  a_ap/c_ap = rank-7 BN coeffs.
            # SumZ8' = a*S1G' + c;     mY = (S1R' + SumZ8')/8
            # SZ2'  = a^2 S2G' + 2ac S1G' + c^2 = a*(a*S2G') + c*(2*SumZ8'-c)
            # SRZ'  = a*SX' + c*S1R'
            # E[Y^2] = (S2R' + 2*SRZ' + SZ2')/64;  var = E[Y^2] - mY^2
            S1G, S2G = stg[:, 0:1], stg[:, 1:2]
            S1R, S2R = stg[:, 2:3], stg[:, 3:4]
            SX = stg[:, 4:5]
            w = spool.tile([H, 12], f32, tag="ywork")
            sz8 = w[:, 0:1]
            my = w[:, 1:2]
            q1 = w[:, 2:3]
            q2 = w[:, 3:4]
            r1_ = w[:, 4:5]
            r2_ = w[:, 5:6]
            sz2 = w[:, 6:7]
            srz = w[:, 7:8]
            w2s = w[:, 8:9]
            msqy = w[:, 9:10]
            vary = w[:, 10:11]
            tmp = w[:, 11:12]
            nc.vector.tensor_scalar(sz8, S1G, a_ap, c_ap, AL.mult, AL.add)
            nc.vector.tensor_scalar(my, sz8, S1R, 0.125, AL.add, AL.mult)
            nc.vector.tensor_tensor(q1, a_ap, S2G, AL.mult)
            nc.vector.tensor_tensor(r1_, a_ap, q1, AL.mult)
            nc.vector.tensor_scalar(q2, sz8, 2.0, c_ap, AL.mult, AL.subtract)
            nc.vector.tensor_tensor(r2_, c_ap, q2, AL.mult)
            nc.vector.tensor_tensor(sz2, r1_, r2_, AL.add)
            nc.vector.tensor_tensor(srz, a_ap, SX, AL.mult)
            nc.vector.tensor_tensor(tmp, c_ap, S1R, AL.mult)
            nc.vector.tensor_tensor(srz, srz, tmp, AL.add)
            nc.vector.tensor_scalar(w2s, srz, 2.0, S2R, AL.mult, AL.add)
            nc.vector.tensor_tensor(w2s, w2s, sz2, AL.add)
            nc.vector.tensor_tensor(msqy, my, my, AL.mult)
            nc.vector.tensor_scalar(vary, w2s, 1.0 / 64.0, msqy,
                                    AL.mult, AL.subtract)
            sdy = spool.tile([H, 5], f32, tag="ycoef")
            sdc = sdy[:, 4:5]
            riy = sdy[:, 0:1]
            ay = sdy[:, 1:2]
            cy = sdy[:, 2:3]
            ay8 = sdy[:, 3:4]
            nc.scalar.activation(sdc, vary, FT.Sqrt, bias=epsc[:])
            nc.vector.reciprocal(riy, sdc)
            nc.vector.tensor_tensor(ay, riy, bnc[:, 2:3], AL.mult)
            nc.vector.tensor_tensor(tmp, my, riy, AL.mult)
            # cy = by - my*ay = (my*riy)*(-gy) + by
            nc.vector.tensor_scalar(cy, tmp, bnc[:, 5:6], bnc[:, 3:4],
                                    AL.mult, AL.add)
            nc.vector.tensor_scalar(ay8, ay, 0.125, None, AL.mult)
            ybn = spool.tile([H, BS], bf16, tag="ybn")
            # per-half apply so the first output matmul starts earlier
            for bt in range(NBT):
                nc.vector.tensor_scalar(ybn[:, bt * 128:(bt + 1) * 128],
                                        yt[:, bt * 128:(bt + 1) * 128],
                                        ay8, cy, AL.mult, AL.add)

            # ---------------- final: relu(relu(Y@W3+b3)+relu(X@W2+b2)) ----
            for bt in range(NBT):
                psA = psmm.tile([128, OUT], f32, tag="mm")
                nc.tensor.matmul(psA[:],
                                 lhsT=ybn[:, bt * 128:(bt + 1) * 128],
                                 rhs=w3[:], start=True, stop=True)
                r1 = spool.tile([128, OUT], f32, tag="r1")
                nc.vector.tensor_tensor(r1[:], psA[:], b3b[:], AL.add)
                r1r = spool.tile([128, OUT], f32, tag="r1r")
                nc.scalar.activation(r1r[:], r1[:], FT.Relu)

                s = spool.tile([128, OUT], f32, tag="s")
                nc.vector.tensor_tensor(s[:], r1r[:], r2rs[bt][:], AL.add)
                of = spool.tile([128, OUT], f32, tag="of")
                nc.scalar.activation(of[:], s[:], FT.Relu)
                nc.scalar.dma_start(OUTd[bt * 128:(bt + 1) * 128, :],
                                    of[:])

    nc.compile()
    return nc


def _bn_sync(nc, tc, dpool, spool, pstr, psd, strow, stw, bnc, onesB,
             ident, ztb, epsc):
    """AllGather per-core [1, stw*128] row stats (single-descriptor DMAs),
    reduce across the 8 cores AND transpose back to per-partition columns
    in one step via contraction-8 matmuls against a 1/B-scaled ones vector
    (f32 operands; tiny N=1 matmuls so the fp32 double-pass is free), then
    compute BN coeffs a, c (s.t. BN(x) = a*x + c) in column layout.

    Returns (a[128,1], c[128,1], E-value columns [128, stw])."""
    from concourse import mybir

    f32 = mybir.dt.float32
    FT = mybir.ActivationFunctionType
    AL = mybir.AluOpType

    W = stw * 128
    src = dpool.tile([1, W], f32, tag=f"ccsrc{stw}")
    dst = dpool.tile([N_CORES, W], f32, tag=f"ccdst{stw}")
    nc.scalar.dma_start(src[:], strow[0:1, 0:W])
    nc.gpsimd.collective_compute(
        "AllGather", AL.bypass, replica_groups=[list(range(N_CORES))],
        ins=[src.opt()], outs=[dst.opt()],
    )
    gath = spool.tile([N_CORES, 8 * 128], f32, tag="gath", bufs=1)
    nst = 5 if stw == 8 else stw
    if stw == 8:
        # split the gather-back so the BN-coeff chain (needs cols 0:256
        # only) overlaps the larger Y-stats transfer
        nc.scalar.dma_start(gath[0:N_CORES, 0:256], dst[:, 0:256])
        nc.scalar.dma_start(gath[0:N_CORES, 256:W], dst[:, 256:W])
    else:
        nc.scalar.dma_start(gath[0:N_CORES, 0:W], dst[:])
    pstc = pstr.tile([128, 8], f32, tag="trb")
    for s in range(nst):
        nc.tensor.matmul(pstc[:, s:s + 1],
                         lhsT=gath[0:N_CORES, s * 128:(s + 1) * 128],
                         rhs=onesB[:], start=True, stop=True)
    st = spool.tile([128, 8], f32, tag="stcol")
    nc.scalar.activation(st[:, 0:2], pstc[:, 0:2], FT.Copy)
    if nst > 2:
        nc.scalar.activation(st[:, 2:nst], pstc[:, 2:nst], FT.Copy)

    # warm-keeper chain 2: cover the coeff-chain window (placed after the
    # reduce matmuls in program order; PE executes in order, so these run
    # between the reduce and the next rank's first real matmul).
    if stw == 2:
        dps = psd.tile([128, 512], f32, tag="warm")
        for _ in range(10):
            nc.tensor.matmul(dps[0:1, :], lhsT=ident[:, 0:1],
                             rhs=ztb[:, 0:512], start=True, stop=True)

    # coeff chain: m = E[x], v = E[x^2] - m^2, rinv = 1/sqrt(v + eps),
    # a = rinv*gz, c = bz - m*a = (m*rinv)*(-gz) + bz
    cf = spool.tile([H, 4], f32, tag="cf")
    msq = cf[:, 0:1]
    v = cf[:, 1:2]
    a = cf[:, 2:3]
    c = cf[:, 3:4]
    rv = spool.tile([H, 3], f32, tag="rv")
    rinv = rv[:, 0:1]
    mr = rv[:, 1:2]
    sd = rv[:, 2:3]
    m = st[:, 0:1]
    ex2 = st[:, 1:2]
    nc.vector.tensor_tensor(msq, m, m, AL.mult)
    nc.vector.tensor_scalar(v, msq, -1.0, ex2, AL.mult, AL.add)
    nc.scalar.activation(sd, v, FT.Sqrt, bias=epsc[:])
    nc.vector.reciprocal(rinv, sd)
    nc.vector.tensor_tensor(a, rinv, bnc[:, 0:1], AL.mult)
    nc.vector.tensor_tensor(mr, m, rinv, AL.mult)
    nc.vector.tensor_scalar(c, mr, bnc[:, 4:5], bnc[:, 1:2],
                            AL.mult, AL.add)
    return a, c, st


def _prep_inputs(X, W1, b1, W2, b2, W3, b3, P, gz, bz, gy, by):
    bf = ml_dtypes.bfloat16
    per_core = []
    P_b = np.ascontiguousarray(P.reshape(RANK, H, QK)).astype(bf)
    W1_b = np.ascontiguousarray(W1.reshape(2, 128, H)).astype(bf)
    W2_b = np.ascontiguousarray(W2.reshape(2, 128, OUT)).astype(bf)
    W3_b = np.ascontiguousarray(W3).astype(bf)
    b1b = np.broadcast_to(b1, (128, H)).astype(np.float32).copy()
    b2b = np.broadcast_to(b2, (128, OUT)).astype(np.float32).copy()
    b3b = np.broadcast_to(b3, (128, OUT)).astype(np.float32).copy()
    bnc = np.stack([gz, bz, gy, by, -gz, -gy], axis=1).astype(np.float32)
    ident = np.eye(128, dtype=np.float32).astype(bf)
    identf = np.eye(128, dtype=np.float32)
    for s in range(N_CORES):
        Xs = X[s * BS:(s + 1) * BS]
        XT = np.ascontiguousarray(Xs.T.reshape(2, 128, BS)).astype(bf)
        per_core.append({
            "XT": XT, "P": P_b, "W1": W1_b, "W2": W2_b, "W3": W3_b,
            "b1b": b1b, "b2b": b2b, "b3b": b3b, "bnc": bnc,
            "ident": ident, "identf": identf,
        })
    return per_core


def kernel(**inputs):
    _ensure_axon_hooks_shim()
    from concourse.bass_utils import run_bass_kernel_spmd

    if "nc" not in _cache:
        _cache["nc"] = _build()
    nc = _cache["nc"]

    in_maps = _prep_inputs(**{k: np.asarray(v) for k, v in inputs.items()})
    res = run_bass_kernel_spmd(nc, in_maps, core_ids=list(range(N_CORES)))
    out = np.concatenate([m["out"] for m in res.results], axis=0)
    return out.astype(np.float32)


if __name__ == "__main__":
    import reference as R

    inputs = {k: np.asarray(v) for k, v in R.setup_inputs().items()}
    got = kernel(**inputs)
    exp = np.asarray(R.reference(**R.setup_inputs()))
    rel = np.linalg.norm(got - exp) / np.linalg.norm(exp)
    print("rel l2:", rel)


# revision 30
# speedup vs baseline: 1.0387x; 1.0387x over previous
"""Distributed Trainium2 kernel for nn_ApaBlock (8 NeuronCores, data-parallel).

Architecture (per core, batch shard of 256 rows):
  Z = relu(X @ W1 + b1)                               (TensorE + DVE/ACT)
  ZT_bcast[p, t, b] = Z^T[t, b]  (replicated over p)  (broadcast DMA, once)
  scan over 8 ranks:
    U^T_t = ZiT * ZT_bcast[t]      (DVE bf16 2x mode, chunked)
    G^T  += P[:,t,:]^T @ U^T_t     (128 accumulating 256-wide matmuls)
    batch stats via ACT accum_out -> PE-transpose to row layout
    cross-core AllGather (sync-BN) in [rows, 128] layout; reduce via
    1/B-scaled ones matmuls (f32, no cast); coeffs with fused Rsqrt
    Zi+1^T = a*G^T + c  fused into the PSUM evacuation
  Y = BN(sum Zi/8) via closed-form global sums (no extra sync);
  out = relu(relu(Y@W3+b3) + relu(X@W2+b2))

v2 perf notes (vs 417us baseline):
  - ztb broadcast is issued as early as possible and with queue priority
    over the P stream; rank-0 U chunks are 16-aligned so they chase the
    broadcast DMA chunk by chunk.
  - The exposed BN-sync window (~15us) gets "warm-keeper" dummy matmuls
    so the PE HAM clock gate never sees a >3.4us idle window; the rank's
    matmul stream then restarts at full clock instead of 1.2GHz.
  - Reduce matmuls run on f32 gather data directly with 1/B pre-scaled
    ones (drops a cast + a scale op); coeff chain uses ACT Rsqrt and
    fused tensor_scalar ops.
  - yt accumulation runs on GPSIMD so DVE is left for the U build
    (DVE is the in-rank pacer at ~17.1us vs PE 16.8us).
  - relu(X@W2+b2) is computed inside rank-0's sync window (free PE time).
"""

import os
import sys
import types

if "/opt/trn_rl_repo" not in sys.path:
    sys.path.insert(0, "/opt/trn_rl_repo")

import numpy as np
import ml_dtypes

N_CORES = 8
B, IN, H, OUT, RANK = 2048, 256, 128, 128, 8
BS = B // N_CORES  # 256 rows per core
NBT = BS // 128  # 2 b-tiles per core
EPS = 1e-5
QK = H * H  # 16384
NCH = 8  # P DMA chunks per rank
TCH = H // NCH  # t's (q-planes) per P chunk (16)
# U-build chunk sizes: tile-aligned so every chunk reads exactly one ztb
# broadcast tile (fine-grained DMA deps let rank 0 chase the broadcast).
# Small chunks at BOTH ends: the first matmul starts one small build
# after zit, and G completes right behind the final DVE build.
UCH = [8, 8, 16, 16, 16, 16, 16, 16, 8, 8]  # sum 128
UMAX = max(UCH)

_cache = {}


def _ensure_axon_hooks_shim():
    """bass_utils imports antenv.axon_hooks when BASS_TRACE is set; the agent
    image lacks it. Provide a null shim so tracing degrades gracefully."""
    try:
        import antenv.axon_hooks  # noqa: F401
        return
    except ImportError:
        pass
    try:
        import antenv  # noqa: F401
    except ImportError:
        return
    mod = types.ModuleType("antenv.axon_hooks")
    _state = {"hook": None}
    mod.set_axon_ntff_profile_hook = lambda h: _state.__setitem__("hook", h)
    mod.get_axon_ntff_profile_hook = lambda: _state["hook"]
    sys.modules["antenv.axon_hooks"] = mod


def _build():
    from concourse import bacc, mybir, tile

    f32 = mybir.dt.float32
    bf16 = mybir.dt.bfloat16
    FT = mybir.ActivationFunctionType
    AL = mybir.AluOpType

    nc = bacc.Bacc("TRN2", target_bir_lowering=False, debug=False,
                   num_devices=N_CORES)

    XTd = nc.declare_dram_parameter("XT", [2, 128, BS], bf16, isOutput=False)
    Pd = nc.declare_dram_parameter("P", [RANK, H, QK], bf16, isOutput=False)
    W1d = nc.declare_dram_parameter("W1", [2, 128, H], bf16, isOutput=False)
    W2d = nc.declare_dram_parameter("W2", [2, 128, OUT], bf16, isOutput=False)
    W3d = nc.declare_dram_parameter("W3", [H, OUT], bf16, isOutput=False)
    B1d = nc.declare_dram_parameter("b1b", [128, H], f32, isOutput=False)
    B2d = nc.declare_dram_parameter("b2b", [128, OUT], f32, isOutput=False)
    B3d = nc.declare_dram_parameter("b3b", [128, OUT], f32, isOutput=False)
    BNCd = nc.declare_dram_parameter("bnc", [H, 6], f32, isOutput=False)
    IDd = nc.declare_dram_parameter("ident", [128, 128], bf16, isOutput=False)
    IDFd = nc.declare_dram_parameter("identf", [128, 128], f32,
                                     isOutput=False)
    OUTd = nc.declare_dram_parameter("out", [BS, OUT], f32, isOutput=True)

    rg = [list(range(N_CORES))]
    CW = TCH * 128  # P columns per chunk (2048)

    with tile.TileContext(nc) as tc:
        with (
            tc.tile_pool(name="const", bufs=1) as cpool,
            tc.tile_pool(name="ppool", bufs=1) as ppool,
            tc.tile_pool(name="upool", bufs=1) as upool,
            tc.tile_pool(name="zit", bufs=2) as zitpool,
            tc.tile_pool(name="small", bufs=2) as spool,
            tc.tile_pool(name="psg", bufs=2, space="PSUM") as psg,
            tc.tile_pool(name="psmm", bufs=2, space="PSUM") as psmm,
            tc.tile_pool(name="pstr", bufs=1, space="PSUM") as pstr,
            tc.tile_pool(name="psd", bufs=1, space="PSUM") as psd,
            tc.tile_pool(name="dram", bufs=4, space="DRAM") as dpool,
        ):
            # Early dummy collectives: the first absorbs cross-core launch
            # skew + ncfw first-call overhead; the second absorbs the slow
            # second-call path so rank-0's real sync runs at steady cost.
            dz = cpool.tile([1, 2 * H], f32, tag="dz")
            nc.vector.memset(dz[:], 0.0)
            dsrc = dpool.tile([1, 2 * H], f32, tag="ccsrcd")
            ddst = dpool.tile([N_CORES, 2 * H], f32, tag="ccdstd")
            nc.scalar.dma_start(dsrc[:], dz[:])
            nc.gpsimd.collective_compute(
                "AllGather", AL.bypass, replica_groups=rg,
                ins=[dsrc.opt()], outs=[ddst.opt()],
            )

            # ---- sync queue: xt (Z dep) -> p0,p1 -> (after dzt) ztb 0..3
            #      -> p2..p7.  scalar queue: w1/b1b/ident (Z deps) ->
            #      dzt-out -> ztb 4..7 -> remaining constants.
            xt = cpool.tile([128, 2 * BS], bf16, tag="xt")
            for c in range(2):
                nc.sync.dma_start(xt[:, c * BS:(c + 1) * BS], XTd[c])

            w1 = cpool.tile([128, 2 * H], bf16, tag="w1")
            for c in range(2):
                nc.scalar.dma_start(w1[:, c * H:(c + 1) * H], W1d[c])
            b1b = cpool.tile([128, H], f32, tag="b1b")
            nc.scalar.dma_start(b1b[:], B1d[:])
            ident = cpool.tile([128, 128], bf16, tag="ident")
            nc.scalar.dma_start(ident[:], IDd[:])
            identf = cpool.tile([128, 128], f32, tag="identf")
            nc.scalar.dma_start(identf[:], IDFd[:])

            p_ch = [ppool.tile([128, CW], bf16, tag=f"p{c}", name=f"p{c}")
                    for c in range(NCH)]
            for c in range(2):
                nc.sync.dma_start(p_ch[c][:], Pd[0][:, c * CW:(c + 1) * CW])

            ones11 = cpool.tile([1, 1], f32, tag="ones11")
            nc.vector.memset(ones11[:], 1.0)
            # cross-core reduce weights with 1/B folded in (st cols become
            # E[x], E[x^2] directly; rank-7 sums become raw/B "primes")
            onesB = cpool.tile([N_CORES, 1], f32, tag="onesB")
            nc.vector.memset(onesB[:], 1.0 / B)
            epsc = cpool.tile([H, 1], f32, tag="epsc")
            nc.vector.memset(epsc[:], EPS)
            epsr = cpool.tile([1, 1], f32, tag="epsr")
            nc.vector.memset(epsr[:], EPS)

            yt = cpool.tile([H, BS], f32, tag="yt")  # Y^T accumulator
            nc.vector.memset(yt[:], 0.0)

            # ---------------- Z = relu(X@W1 + b1) ----------------
            zb = cpool.tile([128, 2 * H], bf16, tag="zb")  # Z, b-partition
            for bt in range(NBT):
                ps = psmm.tile([128, H], f32, tag="mm")
                for c in range(2):
                    nc.tensor.matmul(
                        ps[:],
                        lhsT=xt[:, c * BS + bt * 128: c * BS + (bt + 1) * 128],
                        rhs=w1[:, c * H:(c + 1) * H],
                        start=(c == 0), stop=(c == 1),
                    )
                t0 = spool.tile([128, H], f32, tag="ztmp")
                nc.vector.tensor_tensor(t0[:], ps[:], b1b[:], AL.add)
                nc.scalar.activation(zb[:, bt * H:(bt + 1) * H], t0[:],
                                     FT.Relu)

            # Z^T (q-part, b): initial Zi^T, and the source for ZT_bcast
            zt = cpool.tile([H, BS], bf16, tag="zt")
            for bt in range(NBT):
                pst = pstr.tile([128, 128], bf16, tag="tr")
                nc.tensor.transpose(pst[:],
                                    zb[:, bt * H:(bt + 1) * H], ident[:])
                nc.scalar.activation(zt[:, bt * 128:(bt + 1) * 128],
                                     pst[:], FT.Copy)

            # ZT_bcast[p, (t, b)] = ZT[t, b]: bounce ZT to DRAM, then
            # broadcast-read it back into all 128 partitions, chunked on
            # both queues so rank-0 U-builds chase the chunks.
            dzt = dpool.tile([H, BS], bf16, tag="dzt")
            nc.scalar.dma_start(dzt[:], zt[:])
            ztbs = [cpool.tile([128, TCH * BS], bf16, tag=f"ztb{c}",
                               name=f"ztb{c}")
                    for c in range(NCH)]
            ztb3s = [zc[:].rearrange("p (t b) -> p t b", b=BS)
                     for zc in ztbs]
            # flat contiguous source view: one 8KB descriptor per dest
            # partition instead of 16 strided 512B ones (the [t, b] rows of
            # dzt are contiguous in DRAM) -> ~2x faster broadcast.
            # Interleave the remaining rank-0 P chunks between the even ztb
            # chunks so rank-0's matmuls get P roughly when the matching
            # ztb tile lands (both gate chunk-k progress).
            dztf = dzt[:].rearrange("(o t) b -> o (t b)", o=1)

            def ztb_load(c):
                src = dztf[0:1, c * TCH * BS:(c + 1) * TCH * BS].rearrange(
                    "o n -> o n").broadcast_to((128, TCH * BS))
                eng = nc.sync if c % 2 == 0 else nc.scalar
                eng.dma_start(ztbs[c][:, :], src)

            for c in range(1, NCH, 2):
                ztb_load(c)
            ztb_load(0)
            nc.sync.dma_start(p_ch[2][:], Pd[0][:, 2 * CW:3 * CW])
            ztb_load(2)
            nc.sync.dma_start(p_ch[3][:], Pd[0][:, 3 * CW:4 * CW])
            ztb_load(4)
            nc.sync.dma_start(p_ch[4][:], Pd[0][:, 4 * CW:5 * CW])
            ztb_load(6)
            for c in range(5, NCH):
                nc.sync.dma_start(p_ch[c][:], Pd[0][:, c * CW:(c + 1) * CW])

            # non-critical constants (needed from rank-0 sync onwards)
            w2 = cpool.tile([128, 2 * OUT], bf16, tag="w2")
            for c in range(2):
                nc.scalar.dma_start(w2[:, c * OUT:(c + 1) * OUT], W2d[c])
            w3 = cpool.tile([H, OUT], bf16, tag="w3")
            nc.scalar.dma_start(w3[:], W3d[:])
            b2b = cpool.tile([128, OUT], f32, tag="b2b")
            b3b = cpool.tile([128, OUT], f32, tag="b3b")
            nc.scalar.dma_start(b2b[:], B2d[:])
            nc.scalar.dma_start(b3b[:], B3d[:])
            # bnc cols: [gz, bz, gy, by, -gz, -gy]
            bnc = cpool.tile([H, 6], f32, tag="bnc")
            nc.scalar.dma_start(bnc[:], BNCd[:])

            # ---------------- scan over ranks ----------------
            zit = zt
            gpsum = None
            pend_yt = []
            stg = a_ap = c_ap = None
            for r in range(RANK):
                if r > 0:
                    p_ch = [ppool.tile([128, CW], bf16, tag=f"p{c}",
                                       name=f"p{c}")
                            for c in range(NCH)]
                    for c in range(NCH):
                        nc.sync.dma_start(p_ch[c][:],
                                          Pd[r][:, c * CW:(c + 1) * CW])

                gpsum = psg.tile([128, BS], f32, tag="g")
                t0c = 0
                for j, csz in enumerate(UCH):
                    # U^T chunk: ut[p, t, b] = ZiT[p, b] * ZT[t, b]
                    zin = zit[:].rearrange("p (o b) -> p o b", o=1
                                           ).broadcast_to((128, csz, BS))
                    ut = upool.tile([128, UMAX * BS], bf16, tag=f"u{j % 4}",
                                    name=f"u{j % 4}")
                    ut3 = ut[:, 0:csz * BS].rearrange("p (t b) -> p t b",
                                                      b=BS)
                    zc, zo = t0c // TCH, t0c % TCH
                    nc.vector.tensor_tensor(
                        ut3, zin, ztb3s[zc][:, zo:zo + csz, :], AL.mult)
                    for i in range(csz):
                        t = t0c + i
                        pc, pi = t // TCH, t % TCH
                        nc.tensor.matmul(
                            gpsum[:],
                            lhsT=p_ch[pc][:, pi * 128:(pi + 1) * 128],
                            rhs=ut3[:, i, :],
                            start=(t == 0), stop=(t == H - 1),
                        )
                    t0c += csz

                # batch stats straight from PSUM via ACT accum_out
                last = (r == RANK - 1)
                stw = 8 if last else 2
                stl = spool.tile([H, stw], f32, tag=f"stl{stw}")
                if last:
                    nc.vector.memset(stl[:], 0.0)
                scr = spool.tile([H, BS], bf16, tag="scr")
                if last:
                    gt = spool.tile([H, BS], bf16, tag="gt")
                    nc.scalar.activation(gt[:], gpsum[:], FT.Copy,
                                         accum_out=stl[:, 0:1])
                else:
                    # S1 on DVE (parallel with ACT's Square pass)
                    nc.vector.tensor_reduce(stl[:, 0:1], gpsum[:],
                                            mybir.AxisListType.X, AL.add)
                nc.scalar.activation(scr[:], gpsum[:], FT.Square,
                                     accum_out=stl[:, 1:2])
                if last:
                    # piggyback Y-BN inputs on the final sync: with
                    # R = sum_{i<8} Zi (= yt now) and Zi8 = a*G + c,
                    # SumY and SumY^2 expand in closed form from
                    # [S1G, S2G, S1R, S2R, Sum(R*G)] -- no 9th sync.
                    nc.scalar.activation(scr[:], yt[:], FT.Copy,
                                         accum_out=stl[:, 2:3])
                    nc.scalar.activation(scr[:], yt[:], FT.Square,
                                         accum_out=stl[:, 3:4])
                    scry2 = spool.tile([H, BS], bf16, tag="scry2")
                    nc.vector.tensor_tensor(scry2[:], yt[:], gt[:], AL.mult)
                    nc.scalar.activation(scr[:], scry2[:], FT.Copy,
                                         accum_out=stl[:, 4:5])

                # flatten stats into a single partition-0 row [1, stw*128]
                # (the cross-core bounce then needs only 1 DMA descriptor
                # instead of 16 partition-group descriptors = ~6us saved).
                # f32 lhsT: the fp32 double-pass on these tiny N=128
                # matmuls is cheaper than a serial cast hop.
                strow = spool.tile([1, 8 * 128], f32, tag="strow",
                                   bufs=1)
                for g in range((stw + 3) // 4):
                    ncol = min(4, stw - 4 * g)
                    pstt = pstr.tile([1, 512], f32, tag="trs", name="pstt")
                    for s4 in range(ncol):
                        s = 4 * g + s4
                        nc.tensor.matmul(pstt[0:1, s4 * 128:(s4 + 1) * 128],
                                         lhsT=stl[:, s:s + 1], rhs=identf[:],
                                         start=True, stop=True)
                    nc.scalar.activation(
                        strow[0:1, g * 512:g * 512 + ncol * 128],
                        pstt[0:1, 0:ncol * 128], FT.Copy)

                if r == 0:
                    # relu(X@W2+b2): real work to fill rank-0's sync window
                    r2rs = []
                    for bt in range(NBT):
                        psB = psmm.tile([128, OUT], f32, tag="mm")
                        for c in range(2):
                            nc.tensor.matmul(
                                psB[:],
                                lhsT=xt[:, c * BS + bt * 128:
                                        c * BS + (bt + 1) * 128],
                                rhs=w2[:, c * OUT:(c + 1) * OUT],
                                start=(c == 0), stop=(c == 1),
                            )
                        r2 = spool.tile([128, OUT], f32, tag="r2")
                        nc.vector.tensor_tensor(r2[:], psB[:], b2b[:],
                                                AL.add)
                        r2r = spool.tile([128, OUT], f32, tag=f"r2r{bt}")
                        nc.scalar.activation(r2r[:], r2[:], FT.Relu)
                        r2rs.append(r2r)

                # warm-keeper chain: keep the PE HAM activity window fed
                # through the collective flight so the next rank's matmuls
                # restart at full clock. Gated on scr (right after the
                # rank's real matmuls); ~36x N=512 covers the window.
                if r < RANK - 1:
                    dps = psd.tile([128, 512], f32, tag="warm")
                    nc.tensor.matmul(dps[0:1, 0:256], lhsT=ident[:, 0:1],
                                     rhs=scr[:, 0:BS], start=True,
                                     stop=True)
                    for _ in range(35):
                        nc.tensor.matmul(dps[0:1, :], lhsT=ident[:, 0:1],
                                         rhs=ztbs[0][:, 0:512], start=True,
                                         stop=True)

                # ---- cross-core AllGather of row stats + coeffs ----
                a_ap, c_ap, stg = _bn_sync(nc, tc, dpool, spool, pstr, psd,
                                           strow, stw, bnc, onesB, ident,
                                           ztbs[0], epsc)

                # BN apply fused into the PSUM evacuation:
                # Zi+1^T = a*G^T + c  (then first U chunks; yt flush later)
                zit_next = zitpool.tile([H, BS], bf16, tag="zit")
                nc.vector.tensor_scalar(zit_next[:], gpsum[:], a_ap, c_ap,
                                        AL.mult, AL.add)
                zit = zit_next
                if r < RANK - 1:
                    # warm-keeper chain 3: bridge the zit -> first-chunk
                    # window so the stream restarts at full clock
                    dps3 = psd.tile([128, 512], f32, tag="warm")
                    for _ in range(4):
                        nc.tensor.matmul(dps3[0:1, 0:BS], lhsT=ident[:, 0:1],
                                         rhs=zit_next[:, 0:BS], start=True,
                                         stop=True)
                if r < RANK - 1:
                    pend_yt.append(zit_next)
                else:
                    nc.vector.tensor_tensor(yt[:], yt[:], zit_next[:],
                                            AL.add)

                # deferred yt += Zi_r on GPSIMD (keeps DVE free for U build)
                while pend_yt and r < RANK - 1:
                    nc.gpsimd.tensor_tensor(yt[:], yt[:], pend_yt.pop()[:],
                                            AL.add)

            # ------- Y BN from closed-form global sums (no extra sync) ----
            # stg cols (all already scaled by 1/B): [S1G', S2G', S1R',
            # S2R', SX'].  a_ap/c_ap = rank-7 BN coeffs.
            # SumZ8' = a*S1G' + c;     mY = (S1R' + SumZ8')/8
            # SZ2'  = a^2 S2G' + 2ac S1G' + c^2 = a*(a*S2G') + c*(2*SumZ8'-c)
            # SRZ'  = a*SX' + c*S1R'
            # E[Y^2] = (S2R' + 2*SRZ' + SZ2')/64;  var = E[Y^2] - mY^2
            S1G, S2G = stg[:, 0:1], stg[:, 1:2]
            S1R, S2R = stg[:, 2:3], stg[:, 3:4]
            SX = stg[:, 4:5]
            w = spool.tile([H, 12], f32, tag="ywork")
            sz8 = w[:, 0:1]
            my = w[:, 1:2]
            q1 = w[:, 2:3]
            q2 = w[:, 3:4]
            r1_ = w[:, 4:5]
            r2_ = w[:, 5:6]
            sz2 = w[:, 6:7]
            srz = w[:, 7:8]
            w2s = w[:, 8:9]
            msqy = w[:, 9:10]
            vary = w[:, 10:11]
            tmp = w[:, 11:12]
            nc.vector.tensor_scalar(sz8, S1G, a_ap, c_ap, AL.mult, AL.add)
            nc.vector.tensor_scalar(my, sz8, S1R, 0.125, AL.add, AL.mult)
            nc.vector.tensor_tensor(q1, a_ap, S2G, AL.mult)
            nc.vector.tensor_tensor(r1_, a_ap, q1, AL.mult)
            nc.vector.tensor_scalar(q2, sz8, 2.0, c_ap, AL.mult, AL.subtract)
            nc.vector.tensor_tensor(r2_, c_ap, q2, AL.mult)
            nc.vector.tensor_tensor(sz2, r1_, r2_, AL.add)
            nc.vector.tensor_tensor(srz, a_ap, SX, AL.mult)
            nc.vector.tensor_tensor(tmp, c_ap, S1R, AL.mult)
            nc.vector.tensor_tensor(srz, srz, tmp, AL.add)
            nc.vector.tensor_scalar(w2s, srz, 2.0, S2R, AL.mult, AL.add)
            nc.vector.tensor_tensor(w2s, w2s, sz2, AL.add)
            nc.vector.tensor_tensor(msqy, my, my, AL.mult)
            nc.vector.tensor_scalar(vary, w2s, 1.0 / 64.0, msqy,
                                    AL.mult, AL.subtract)
            sdy = spool.tile([H, 5], f32, tag="ycoef")
            sdc = sdy[:, 4:5]
            riy = sdy[:, 0:1]
            ay = sdy[:, 1:2]
            cy = sdy[:, 2:3]
            ay8 = sdy[:, 3:4]
            nc.scalar.activation(sdc, vary, FT.Sqrt, bias=epsc[:])
            nc.vector.reciprocal(riy, sdc)
            nc.vector.tensor_tensor(ay, riy, bnc[:, 2:3], AL.mult)
            nc.vector.tensor_tensor(tmp, my, riy, AL.mult)
            # cy = by - my*ay = (my*riy)*(-gy) + by
            nc.vector.tensor_scalar(cy, tmp, bnc[:, 5:6], bnc[:, 3:4],
                                    AL.mult, AL.add)
            nc.vector.tensor_scalar(ay8, ay, 0.125, None, AL.mult)
            ybn = spool.tile([H, BS], bf16, tag="ybn")
            # per-half apply so the first output matmul starts earlier
            for bt in range(NBT):
                nc.vector.tensor_scalar(ybn[:, bt * 128:(bt + 1) * 128],
                                        yt[:, bt * 128:(bt + 1) * 128],
                                        ay8, cy, AL.mult, AL.add)

            # ---------------- final: relu(relu(Y@W3+b3)+relu(X@W2+b2)) ----
            for bt in range(NBT):
                psA = psmm.tile([128, OUT], f32, tag="mm")
                nc.tensor.matmul(psA[:],
                                 lhsT=ybn[:, bt * 128:(bt + 1) * 128],
                                 rhs=w3[:], start=True, stop=True)
                r1 = spool.tile([128, OUT], f32, tag="r1")
                nc.vector.tensor_tensor(r1[:], psA[:], b3b[:], AL.add)
                r1r = spool.tile([128, OUT], f32, tag="r1r")
                nc.scalar.activation(r1r[:], r1[:], FT.Relu)

                s = spool.tile([128, OUT], f32, tag="s")
                nc.vector.tensor_tensor(s[:], r1r[:], r2rs[bt][:], AL.add)
                of = spool.tile([128, OUT], f32, tag="of")
                nc.scalar.activation(of[:], s[:], FT.Relu)
                nc.scalar.dma_start(OUTd[bt * 128:(bt + 1) * 128, :],
                                    of[:])

    nc.compile()
    return nc


def _bn_sync(nc, tc, dpool, spool, pstr, psd, strow, stw, bnc, onesB,
             ident, ztb, epsc):
    """AllGather per-core [1, stw*128] row stats (single-descriptor DMAs),
    reduce across the 8 cores AND transpose back to per-partition columns
    in one step via contraction-8 matmuls against a 1/B-scaled ones vector
    (f32 operands; tiny N=1 matmuls so the fp32 double-pass is free), then
    compute BN coeffs a, c (s.t. BN(x) = a*x + c) in column layout.

    Returns (a[128,1], c[128,1], E-value columns [128, stw])."""
    from concourse import mybir

    f32 = mybir.dt.float32
    FT = mybir.ActivationFunctionType
    AL = mybir.AluOpType

    W = stw * 128
    src = dpool.tile([1, W], f32, tag=f"ccsrc{stw}")
    dst = dpool.tile([N_CORES, W], f32, tag=f"ccdst{stw}")
    nc.scalar.dma_start(src[:], strow[0:1, 0:W])
    nc.gpsimd.collective_compute(
        "AllGather", AL.bypass, replica_groups=[list(range(N_CORES))],
        ins=[src.opt()], outs=[dst.opt()],
    )
    gath = spool.tile([N_CORES, 8 * 128], f32, tag="gath", bufs=1)
    nst = 5 if stw == 8 else stw
    if stw == 8:
        # split the gather-back so the BN-coeff chain (needs cols 0:256
        # only) overlaps the larger Y-stats transfer
        nc.scalar.dma_start(gath[0:N_CORES, 0:256], dst[:, 0:256])
        nc.scalar.dma_start(gath[0:N_CORES, 256:W], dst[:, 256:W])
    else:
        nc.scalar.dma_start(gath[0:N_CORES, 0:W], dst[:])
    pstc = pstr.tile([128, 8], f32, tag="trb")
    for s in range(nst):
        nc.tensor.matmul(pstc[:, s:s + 1],
                         lhsT=gath[0:N_CORES, s * 128:(s + 1) * 128],
                         rhs=onesB[:], start=True, stop=True)
    st = spool.tile([128, 8], f32, tag="stcol")
    nc.scalar.activation(st[:, 0:2], pstc[:, 0:2], FT.Copy)
    if nst > 2:
        nc.scalar.activation(st[:, 2:nst], pstc[:, 2:nst], FT.Copy)

    # warm-keeper chain 2: cover the coeff-chain window (placed after the
    # reduce matmuls in program order; PE executes in order, so these run
    # between the reduce and the next rank's first real matmul).
    if stw == 2:
        dps = psd.tile([128, 512], f32, tag="warm")
        for _ in range(10):
            nc.tensor.matmul(dps[0:1, :], lhsT=ident[:, 0:1],
                             rhs=ztb[:, 0:512], start=True, stop=True)

    # coeff chain: m = E[x], v = E[x^2] - m^2, rinv = 1/sqrt(v + eps),
    # a = rinv*gz, c = bz - m*a = (m*rinv)*(-gz) + bz
    cf = spool.tile([H, 4], f32, tag="cf")
    msq = cf[:, 0:1]
    v = cf[:, 1:2]
    a = cf[:, 2:3]
    c = cf[:, 3:4]
    rv = spool.tile([H, 3], f32, tag="rv")
    rinv = rv[:, 0:1]
    mr = rv[:, 1:2]
    sd = rv[:, 2:3]
    m = st[:, 0:1]
    ex2 = st[:, 1:2]
    nc.vector.tensor_tensor(msq, m, m, AL.mult)
    nc.vector.tensor_scalar(v, msq, -1.0, ex2, AL.mult, AL.add)
    nc.scalar.activation(sd, v, FT.Sqrt, bias=epsc[:])
    nc.vector.reciprocal(rinv, sd)
    nc.vector.tensor_tensor(a, rinv, bnc[:, 0:1], AL.mult)
    nc.vector.tensor_tensor(mr, m, rinv, AL.mult)
    nc.vector.tensor_scalar(c, mr, bnc[:, 4:5], bnc[:, 1:2],
                            AL.mult, AL.add)
    return a, c, st


def _prep_inputs(X, W1, b1, W2, b2, W3, b3, P, gz, bz, gy, by):
    bf = ml_dtypes.bfloat16
    per_core = []
    P_b = np.ascontiguousarray(P.reshape(RANK, H, QK)).astype(bf)
    W1_b = np.ascontiguousarray(W1.reshape(2, 128, H)).astype(bf)
    W2_b = np.ascontiguousarray(W2.reshape(2, 128, OUT)).astype(bf)
    W3_b = np.ascontiguousarray(W3).astype(bf)
    b1b = np.broadcast_to(b1, (128, H)).astype(np.float32).copy()
    b2b = np.broadcast_to(b2, (128, OUT)).astype(np.float32).copy()
    b3b = np.broadcast_to(b3, (128, OUT)).astype(np.float32).copy()
    bnc = np.stack([gz, bz, gy, by, -gz, -gy], axis=1).astype(np.float32)
    ident = np.eye(128, dtype=np.float32).astype(bf)
    identf = np.eye(128, dtype=np.float32)
    for s in range(N_CORES):
        Xs = X[s * BS:(s + 1) * BS]
        XT = np.ascontiguousarray(Xs.T.reshape(2, 128, BS)).astype(bf)
        per_core.append({
            "XT": XT, "P": P_b, "W1": W1_b, "W2": W2_b, "W3": W3_b,
            "b1b": b1b, "b2b": b2b, "b3b": b3b, "bnc": bnc,
            "ident": ident, "identf": identf,
        })
    return per_core


def kernel(**inputs):
    _ensure_axon_hooks_shim()
    from concourse.bass_utils import run_bass_kernel_spmd

    if "nc" not in _cache:
        _cache["nc"] = _build()
    nc = _cache["nc"]

    in_maps = _prep_inputs(**{k: np.asarray(v) for k, v in inputs.items()})
    res = run_bass_kernel_spmd(nc, in_maps, core_ids=list(range(N_CORES)))
    out = np.concatenate([m["out"] for m in res.results], axis=0)
    return out.astype(np.float32)


if __name__ == "__main__":
    import reference as R

    inputs = {k: np.asarray(v) for k, v in R.setup_inputs().items()}
    got = kernel(**inputs)
    exp = np.asarray(R.reference(**R.setup_inputs()))
    rel = np.linalg.norm(got - exp) / np.linalg.norm(exp)
    print("rel l2:", rel)
